# revision 6
# baseline (speedup 1.0000x reference)
"""Trainium2 Bass kernel for nn_DNN_sym_new (gnn_message_passing).

Computation: per-atom type-conditioned MLP embedding (3 -> 32 -> 64, LeakyReLU)
followed by permutation-invariant aggregation d = (g.T @ x) [64,3], then a small
fitting net 192 -> 256 -> 256 -> 3.

Strategy (8 cores, data-parallel over the 1M-atom axis):
 - Host: round-robin shard atoms per (core, type); pad each (core,type) segment
   with zero-atoms to a common size S (zero atoms contribute g(0) (x) 0 = 0).
 - Device per core (SPMD), all matmul operands bf16 (PSUM accumulates fp32):
     L1:  one K=16 matmul per 512-column phase with a block-diagonal stationary
          [W0[t]; b0[t]] over 4 type-blocks (ones-row in the x stream carries
          the bias) -> PSUM [128, 512] = 4 type-blocks x 32 feats.
     h:   ACT LeakyReLU drain PSUM -> SBUF bf16 (feat-major), 1024-col ops.
     L2:  per 128-atom chunk, transition to atom-major: lhsT = hT-slice
          [32,128] (atoms become output partitions), rhs = W1[t] [32,64] ->
          z PSUM [128 atoms, 64]; bias b1 via a K=1 ones-matmul per bank.
     g:   ACT LeakyReLU drain -> SBUF bf16 [128, 1024].
     agg: lhsT = x-atom-chunk [128, 3] (3-column weight load), rhs = g chunk
          -> accumulate into one persistent PSUM [3, 64] (fp32).
 - Host: sum the 8 partial [3,64] outputs, run the tiny fitting net in numpy.
"""

import numpy as np
import ml_dtypes
from contextlib import ExitStack

N_CORES = 8
T = 4
E0, E1 = 32, 64
SLOPE = 0.01
CW = 512        # atoms per type per phase (one PSUM bank of fp32 columns)
GRP = 8         # phases per DMA group
BF = ml_dtypes.bfloat16

_BUILD_CACHE = {}


def _build_bass(S, use_lrelu=True):
    """Build + compile the per-core Bass module for segment size S."""
    key = (S, use_lrelu)
    if key in _BUILD_CACHE:
        return _BUILD_CACHE[key]

    import concourse.bass as bass  # noqa: F401
    import concourse.tile as tile
    from concourse import bacc, mybir

    f32 = mybir.dt.float32
    bf16 = mybir.dt.bfloat16
    AF = mybir.ActivationFunctionType
    act_fn = AF.Lrelu if use_lrelu else AF.Relu

    nc = bacc.Bacc("TRN2", target_bir_lowering=False, debug=False,
                   num_devices=N_CORES)

    n_phase = S // CW
    assert n_phase % GRP == 0
    n_grp = n_phase // GRP
    n_chunk = S // 128  # atom-major chunks per type

    xd = nc.dram_tensor("xd", [16, S], bf16, kind="ExternalInput").ap()
    xa = nc.dram_tensor("xa", [128, T * n_chunk * 3], bf16,
                        kind="ExternalInput").ap()
    l1w = nc.dram_tensor("l1w", [16, 128], bf16, kind="ExternalInput").ap()
    w1aug = nc.dram_tensor("w1aug", [33, 4 * E1], bf16,
                           kind="ExternalInput").ap()
    part = nc.dram_tensor("part", [3, E1], f32, kind="ExternalOutput").ap()

    with tile.TileContext(nc) as tc:
        with ExitStack() as ctx:
            consts = ctx.enter_context(tc.tile_pool(name="consts", bufs=1))
            xpool = ctx.enter_context(tc.tile_pool(name="xp", bufs=2))
            xapool = ctx.enter_context(tc.tile_pool(name="xap", bufs=2))
            hpool = ctx.enter_context(tc.tile_pool(name="hp", bufs=2))
            gpool = ctx.enter_context(tc.tile_pool(name="gp", bufs=2))
            l1ps = ctx.enter_context(
                tc.tile_pool(name="l1ps", bufs=1, space="PSUM"))
            zps = ctx.enter_context(
                tc.tile_pool(name="zps", bufs=2, space="PSUM"))
            aggp = ctx.enter_context(
                tc.tile_pool(name="aggp", bufs=1, space="PSUM"))
            outp = ctx.enter_context(tc.tile_pool(name="outp", bufs=1))

            l1w_sb = consts.tile([16, 128], bf16)
            nc.sync.dma_start(l1w_sb[:], l1w[:])
            w1_sb = consts.tile([33, 4 * E1], bf16)
            nc.sync.dma_start(w1_sb[:], w1aug[:])
            # persistent per-type hT tiles (double-buffered); row 32 = ones
            htj = [[consts.tile([33, 2 * CW], bf16, name=f"htj_{b}_{j}", tag=f"htj_{b}_{j}")
                    for j in range(T)] for b in range(2)]
            for b in range(2):
                for j in range(T):
                    nc.gpsimd.memset(htj[b][j][32:33, :], 1.0)

            agg = aggp.tile([3, E1], f32)
            xa_v = xa.rearrange("p (j g c) -> p j g c", j=T, c=3)

            first_agg = True
            for gi in range(n_grp):
                xt = xpool.tile([16, GRP * CW], bf16)
                nc.sync.dma_start(
                    xt[:], xd[:, gi * GRP * CW:(gi + 1) * GRP * CW])
                xat = xapool.tile([128, T, 4 * GRP, 3], bf16)
                nc.sync.dma_start(
                    xat[:, :, :, :],
                    xa_v[:, :, gi * 4 * GRP:(gi + 1) * 4 * GRP, :])

                for pp in range(GRP // 2):  # phase pairs
                    l1p = l1ps.tile([128, 2 * CW], f32)
                    for s in range(2):
                        nc.tensor.matmul(
                            l1p[:, s * CW:(s + 1) * CW], l1w_sb[:, :],
                            xt[:, (2 * pp + s) * CW:(2 * pp + s + 1) * CW],
                            start=True, stop=True)
                    ht = hpool.tile([128, 2 * CW], bf16)
                    nc.scalar.activation(ht[:], l1p[:], act_fn, alpha=SLOPE)
                    hb = htj[pp % 2]
                    for j in range(T):
                        nc.vector.tensor_copy(hb[j][0:32, :],
                                              ht[32 * j:32 * (j + 1), :])

                    for s in range(2):  # the two phases in the pair
                        zp = zps.tile([128, 2 * CW], f32)
                        for q in range(16):
                            j = q // 4
                            u = q % 4
                            nc.tensor.matmul(
                                zp[:, 64 * q:64 * (q + 1)],
                                hb[j][0:33,
                                      s * CW + 128 * u:s * CW + 128 * (u + 1)],
                                w1_sb[0:33, E1 * j:E1 * (j + 1)],
                                start=True, stop=True)
                        g = gpool.tile([128, 2 * CW], bf16)
                        nc.scalar.activation(g[:], zp[:], act_fn, alpha=SLOPE)
                        for q in range(16):
                            j = q // 4
                            u = q % 4
                            uu = (2 * pp + s) * 4 + u  # u-chunk within group
                            nc.tensor.matmul(
                                agg[:, :],
                                xat[:, j, uu, :],
                                g[:, 64 * q:64 * (q + 1)],
                                start=first_agg, stop=False,
                                skip_group_check=True)
                            first_agg = False

            res = outp.tile([3, E1], f32)
            nc.scalar.copy(res[:], agg[:])
            nc.sync.dma_start(part[:], res[:])

    nc.compile()
    _BUILD_CACHE[key] = nc
    return nc


def _lrelu(v):
    return np.where(v > 0, v, SLOPE * v).astype(np.float32)


def _prep_inputs(x, atom_list, W0, b0, W1, b1):
    """Host-side shard + layout construction. Returns (S, in_maps)."""
    x = np.asarray(x, dtype=np.float32)
    atom_list = np.asarray(atom_list)

    # per-(core,type) atom index lists, round-robin over each type's atoms
    idx = [[None] * T for _ in range(N_CORES)]
    max_n = 0
    for t in range(T):
        it = np.flatnonzero(atom_list == t)
        for c in range(N_CORES):
            ic = it[c::N_CORES]
            idx[c][t] = ic
            max_n = max(max_n, len(ic))
    SGRAN = CW * GRP
    S = ((max_n + SGRAN - 1) // SGRAN) * SGRAN
    n_chunk = S // 128

    # constants (shared by all cores)
    l1w = np.zeros((16, 128), np.float32)
    for j in range(T):
        l1w[4 * j:4 * j + 3, 32 * j:32 * (j + 1)] = W0[j]
        l1w[4 * j + 3, 32 * j:32 * (j + 1)] = b0[j]
    w1aug = np.zeros((33, 4 * E1), np.float32)
    for j in range(T):
        w1aug[0:32, E1 * j:E1 * (j + 1)] = W1[j]
        w1aug[32, E1 * j:E1 * (j + 1)] = b1[j]

    l1w = l1w.astype(BF)
    w1aug = w1aug.astype(BF)

    in_maps = []
    for c in range(N_CORES):
        xd = np.zeros((16, S), np.float32)
        xa = np.zeros((128, T * n_chunk * 3), np.float32)
        for j in range(T):
            ic = idx[c][j]
            n = len(ic)
            xs = x[ic]  # [n, 3]
            xd[4 * j:4 * j + 3, :n] = xs.T
            xd[4 * j + 3, :] = 1.0
            xs_pad = np.zeros((n_chunk * 128, 3), np.float32)
            xs_pad[:n] = xs
            blk = xs_pad.reshape(n_chunk, 128, 3).transpose(1, 0, 2)
            xa[:, j * n_chunk * 3:(j + 1) * n_chunk * 3] = blk.reshape(128, -1)
        in_maps.append({
            "xd": xd.astype(BF), "xa": xa.astype(BF),
            "l1w": l1w, "w1aug": w1aug,
        })
    return S, in_maps


def kernel(x, atom_list, W0, b0, W1, b1, Wf1, bf1, Wf2, bf2, Wo, bo):
    from concourse.bass_utils import run_bass_kernel_spmd

    W0 = np.asarray(W0, np.float32)
    b0 = np.asarray(b0, np.float32)
    W1 = np.asarray(W1, np.float32)
    b1 = np.asarray(b1, np.float32)

    S, in_maps = _prep_inputs(x, atom_list, W0, b0, W1, b1)
    nc = _build_bass(S)
    res = run_bass_kernel_spmd(nc, in_maps, core_ids=list(range(N_CORES)))

    partial = np.zeros((3, E1), np.float64)
    for r in res.results:
        partial += r["part"].astype(np.float64)

    # partial[c, e] = sum_a x[a, c] * g[a, e]  ->  d[e*3+c]
    d = partial.T.astype(np.float32).reshape(-1)  # [192]

    d = _lrelu(d @ np.asarray(Wf1, np.float32) + np.asarray(bf1, np.float32))
    d = _lrelu(d @ np.asarray(Wf2, np.float32) + np.asarray(bf2, np.float32))
    out = d @ np.asarray(Wo, np.float32) + np.asarray(bo, np.float32)
    return out.astype(np.float32)
